# revision 1
# baseline (speedup 1.0000x reference)
"""Kernel-target-alignment loss on 8 TRN2 NeuronCores.

Math: Xs = X*sqrt(params); d2_ij = ||Xs_i - Xs_j||^2; K = exp(-d2) (diag == 1);
kta = sum(K*tt^T) / (N*sqrt(sum(K*K)));  return -kta.

Strategy (row-sharded across 8 cores, 1024 rows each):
  A_ij = 2*sum_d p_d x_i x_j - sq_i - sq_j  (= -d2), built per [128,1024] tile as
  one augmented fp32 matmul (K=65: [2p*X^T ; ones] x [X^T ; -sq]) giving
  2G - sq_j in PSUM; the -sq_i row term rides the ACT exp bias (a [128,1] column
  obtained by a K=1 PE matmul "transpose" of the -sq row, so row and column sq
  are bit-identical).  E = exp(A) in bf16.  Two fused DVE tensor_tensor_reduce
  ops per tile give row-sums of E*E (for sum K^2) and E*t_col (for t^T K t).
  Per-core partials return to the host for the final scalar combine.  No sqrt
  anywhere (lhs carries p*x, rhs carries x), so A_ii is fp32-exact ~0 and
  bf16(exp(A_ii)) == 1.0 exactly, matching the reference's unit diagonal.
"""

import numpy as np

import concourse.bass as bass
import concourse.bacc as bacc
import concourse.tile as tile
import concourse.mybir as mybir
from concourse.bass_utils import run_bass_kernel_spmd

N = 8192
D = 64
NCORES = 8
RPC = N // NCORES          # 1024 rows per core
NRB = RPC // 128           # 8 row blocks of 128 rows
CW = 1024                  # column tile width (2 PSUM banks fp32)
NCT = N // CW              # 8 column tiles
NSLOT = NRB * NCT          # 64 accumulator slots per core

F32 = mybir.dt.float32
BF16 = mybir.dt.bfloat16


def _ap(tensor, ap):
    return bass.AP(tensor=tensor, offset=0, ap=ap)


def build_kernel(variant="stt"):
    nc = bacc.Bacc("TRN2", target_bir_lowering=False)

    xt_d = nc.dram_tensor("xt", [D, N], F32, kind="ExternalInput")
    lt_d = nc.dram_tensor("lt", [D, RPC], F32, kind="ExternalInput")
    t_d = nc.dram_tensor("t", [N], F32, kind="ExternalInput")
    params_d = nc.dram_tensor("params", [D], F32, kind="ExternalInput")
    rsq_d = nc.dram_tensor("rsq_scratch", [RPC], F32)
    s1o_d = nc.dram_tensor("s1o", [128, NSLOT], F32, kind="ExternalOutput")
    s2o_d = nc.dram_tensor("s2o", [128, NSLOT], F32, kind="ExternalOutput")

    with tile.TileContext(nc) as tc:
        with (
            tc.tile_pool(name="const", bufs=1) as cpool,
            tc.tile_pool(name="ztiles", bufs=3) as zpool,
            tc.tile_pool(name="etile", bufs=4) as epool,
            tc.tile_pool(name="scratch", bufs=4) as spool,
            tc.tile_pool(name="mmpsum", bufs=2, space="PSUM") as mpool,
            tc.tile_pool(name="setpsum", bufs=3, space="PSUM") as qpool,
        ):
            # ---- persistent SBUF tensors -------------------------------------
            R = cpool.tile([D + 1, N], F32, tag="R")       # [x^T ; -sq]
            L = cpool.tile([D + 1, RPC], F32, tag="L")     # local [2p*x^T ; ones]
            lt_sb = cpool.tile([D, RPC], F32, tag="ltsb")  # local x^T slice
            sqloc = cpool.tile([1, RPC], F32, tag="sqloc")  # local -sq row
            xs1l = cpool.tile([D, RPC], F32, tag="xs1l")   # local p*x^T
            xs1 = cpool.tile([D, N], F32, tag="xs1")       # p * x^T
            tcol = cpool.tile([128, N], BF16, tag="tcol")  # t broadcast to 128 parts
            tcolf = cpool.tile([128, N], F32, tag="tcolf")
            psb = cpool.tile([D, 1], F32, tag="psb")
            p2sb = cpool.tile([D, 1], F32, tag="p2sb")
            neg1 = cpool.tile([D, 1], F32, tag="neg1")
            rsqn = cpool.tile([128, NRB], F32, tag="rsqn")
            rsqn2 = cpool.tile([128, NRB], F32, tag="rsqn2")
            s1acc = cpool.tile([128, NSLOT], F32, tag="s1acc")
            s2acc = cpool.tile([128, NSLOT], F32, tag="s2acc")

            # ---- setup -------------------------------------------------------
            for s in range(16):
                sl = slice(s * 512, (s + 1) * 512)
                nc.sync.dma_start(out=R[0:D, sl], in_=xt_d[:, sl])
            nc.gpsimd.dma_start(out=psb[:, :], in_=_ap(params_d, [[1, D], [0, 1]]))
            nc.sync.dma_start(out=lt_sb[:, :], in_=lt_d[:, :])
            for s in range(8):
                sl = slice(s * (N // 8), (s + 1) * (N // 8))
                nc.sync.dma_start(
                    out=tcolf[:, sl],
                    in_=bass.AP(tensor=t_d, offset=s * (N // 8), ap=[[0, 128], [1, N // 8]]),
                )
            nc.vector.tensor_scalar_mul(p2sb[:, :], psb[:, :], 2.0)
            nc.vector.memset(neg1[:, :], -1.0)
            nc.gpsimd.memset(L[D : D + 1, :], 1.0)
            nc.vector.tensor_scalar_mul(L[0:D, :], lt_sb[:, :], p2sb[:, :])
            nc.vector.tensor_scalar_mul(xs1l[:, :], lt_sb[:, :], psb[:, :])

            # xs1 = p*x^T  (sliced for pipelining)
            for s in range(8):
                sl = slice(s * (N // 8), (s + 1) * (N // 8))
                nc.vector.tensor_scalar_mul(xs1[:, sl], R[0:D, sl], psb[:, :])

            # col-layout -sq (R row D) via PE partition-reduce of z = xs1 * x
            for s in range(16):
                sl = slice(s * 512, (s + 1) * 512)
                zt = zpool.tile([D, 512], F32, tag="z")
                nc.vector.tensor_mul(zt[:, :], xs1[:, sl], R[0:D, sl])
                q = qpool.tile([128, 512], F32, tag="qps")
                nc.tensor.matmul(
                    q[0:1, :], neg1[:, :], zt[:, :], start=True, stop=True
                )
                nc.scalar.copy(out=R[D : D + 1, sl], in_=q[0:1, :])

            # local -sq row for this core's rows (same fp ops as column path)
            for s in range(RPC // 512):
                sl = slice(s * 512, (s + 1) * 512)
                zt = zpool.tile([D, 512], F32, tag="z")
                nc.vector.tensor_mul(zt[:, :], xs1l[:, sl], lt_sb[:, sl])
                q = qpool.tile([128, 512], F32, tag="qps")
                nc.tensor.matmul(
                    q[0:1, :], neg1[:, :], zt[:, :], start=True, stop=True
                )
                nc.scalar.copy(out=sqloc[:, sl], in_=q[0:1, :])

            # row-layout -sq for the exp bias: bounce through DRAM so the
            # [1, RPC] row can be re-read as a [128, NRB] partition-major tile:
            # rsqn[p, rb] = sqloc[0, rb*128 + p]
            nc.gpsimd.dma_start(out=_ap(rsq_d, [[0, 1], [1, RPC]]), in_=sqloc[:, :])
            nc.gpsimd.dma_start(out=rsqn[:, :], in_=_ap(rsq_d, [[1, 128], [128, NRB]]))
            nc.vector.tensor_scalar_mul(rsqn2[:, :], rsqn[:, :], 2.0)

            # tcol: cast broadcast t to bf16
            for s in range(8):
                sl = slice(s * (N // 8), (s + 1) * (N // 8))
                nc.vector.tensor_copy(out=tcol[:, sl], in_=tcolf[:, sl])

            if variant == "nott":
                nc.vector.memset(s1acc[:, :], 0.0)
                nc.vector.memset(s2acc[:, :], 0.0)
            # ---- main loop ---------------------------------------------------
            for rb in range(NRB):
                lhsT = L[:, rb * 128 : (rb + 1) * 128]
                bias = rsqn[:, rb : rb + 1]
                for ct in range(NCT):
                    slot = rb * NCT + ct
                    mm = mpool.tile([128, CW], F32, tag="mm")
                    for j in range(CW // 512):
                        sl = slice(ct * CW + j * 512, ct * CW + (j + 1) * 512)
                        nc.tensor.matmul(
                            mm[:, j * 512 : (j + 1) * 512],
                            lhsT,
                            R[:, sl],
                            start=True,
                            stop=True,
                        )
                    EDT = F32 if variant == "ttrf32" else BF16
                    E = epool.tile([128, CW], EDT, tag="E")
                    if variant == "noexp":
                        nc.scalar.copy(out=E[:, :], in_=mm[:, :])
                    else:
                        nc.scalar.activation(
                            out=E[:, :], in_=mm[:, :],
                            func=mybir.ActivationFunctionType.Exp,
                            bias=bias, scale=1.0,
                        )
                    if variant == "nott":
                        continue
                    sc1 = spool.tile([128, CW], EDT, tag="sc1")
                    tcol_in = tcolf if variant == "ttrf32" else tcol
                    if variant in ("stt", "g1", "act2"):
                        if variant == "g1":
                            nc.gpsimd.scalar_tensor_tensor(
                                out=sc1[:, :], in0=E[:, :], scalar=1.0, in1=E[:, :],
                                op0=mybir.AluOpType.mult, op1=mybir.AluOpType.mult,
                                accum_out=s1acc[:, slot : slot + 1],
                            )
                        elif variant == "act2":
                            nc.scalar.activation(
                                out=sc1[:, :], in_=mm[:, :],
                                func=mybir.ActivationFunctionType.Exp,
                                bias=rsqn2[:, rb : rb + 1], scale=2.0,
                                accum_out=s1acc[:, slot : slot + 1],
                            )
                        else:
                            nc.vector.scalar_tensor_tensor(
                                out=sc1[:, :], in0=E[:, :], scalar=1.0, in1=E[:, :],
                                op0=mybir.AluOpType.mult, op1=mybir.AluOpType.mult,
                                accum_out=s1acc[:, slot : slot + 1],
                            )
                        sc2 = spool.tile([128, CW], EDT, tag="sc2")
                        nc.vector.scalar_tensor_tensor(
                            out=sc2[:, :], in0=E[:, :], scalar=1.0,
                            in1=tcol_in[:, ct * CW : (ct + 1) * CW],
                            op0=mybir.AluOpType.mult, op1=mybir.AluOpType.mult,
                            accum_out=s2acc[:, slot : slot + 1],
                        )
                    else:
                        nc.vector.tensor_tensor_reduce(
                            out=sc1[:, :], in0=E[:, :], in1=E[:, :],
                            scale=1.0, scalar=0.0,
                            op0=mybir.AluOpType.mult, op1=mybir.AluOpType.add,
                            accum_out=s1acc[:, slot : slot + 1],
                        )
                        sc2 = spool.tile([128, CW], EDT, tag="sc2")
                        nc.vector.tensor_tensor_reduce(
                            out=sc2[:, :], in0=E[:, :],
                            in1=tcol_in[:, ct * CW : (ct + 1) * CW],
                            scale=1.0, scalar=0.0,
                            op0=mybir.AluOpType.mult, op1=mybir.AluOpType.add,
                            accum_out=s2acc[:, slot : slot + 1],
                        )

            nc.sync.dma_start(out=s1o_d[:, :], in_=s1acc[:, :])
            nc.sync.dma_start(out=s2o_d[:, :], in_=s2acc[:, :])

    nc.compile()
    return nc


_NC_CACHE = None


def make_in_maps(X, target, params):
    X = np.ascontiguousarray(X, dtype=np.float32)
    target = np.ascontiguousarray(target, dtype=np.float32)
    params = np.ascontiguousarray(params, dtype=np.float32)
    xt = np.ascontiguousarray(X.T)
    return [
        {
            "xt": xt,
            "lt": np.ascontiguousarray(xt[:, c * RPC : (c + 1) * RPC]),
            "t": target,
            "params": params,
        }
        for c in range(NCORES)
    ]


def kernel(X, target, params):
    global _NC_CACHE
    X = np.ascontiguousarray(X, dtype=np.float32)
    target = np.ascontiguousarray(target, dtype=np.float32)
    params = np.ascontiguousarray(params, dtype=np.float32)

    in_maps = make_in_maps(X, target, params)

    if _NC_CACHE is None:
        _NC_CACHE = build_kernel()
    res = run_bass_kernel_spmd(_NC_CACHE, in_maps, core_ids=list(range(NCORES)))

    s1 = 0.0
    s2 = 0.0
    for c in range(NCORES):
        s1o = res.results[c]["s1o"]  # [128, NSLOT]
        s2o = res.results[c]["s2o"]  # [128, NSLOT]
        s1 += float(s1o.sum())
        u = s2o.reshape(128, NRB, NCT).sum(axis=2)              # [128, NRB]
        tb = target[c * RPC : (c + 1) * RPC].reshape(NRB, 128)  # [NRB, 128]
        s2 += float(np.sum(u.T * tb))

    val = -s2 / (N * np.sqrt(s1))
    return np.array(val, dtype=np.float32)



# revision 5
# speedup vs baseline: 2.6563x; 2.6563x over previous
"""Kernel-target-alignment loss on 8 TRN2 NeuronCores (v2).

Math: Xs = X*sqrt(p); d2_ij = ||Xs_i - Xs_j||^2; K = exp(-d2) (diag := 1);
kta = sum(K*tt^T) / (N*sqrt(sum(K*K)));  return -kta.

v2 strategy:
  * Exact diagonal on host: S2 = N + offdiag, S1 = sum(t^2) + offdiag.
    Device computes only off-diagonal sums; the K diagonal is suppressed by
    adding -BIG to A_ii via a second (identity-weights) matmul on diagonal
    tiles, so no bit-exact sq path is needed and everything runs in bf16.
  * Triangle-of-work: by symmetry only ~half the N^2 pairs are computed.
    Row-block r (512 rows) pairs with column blocks r..r+8 (mod 16); core c
    owns row blocks {c, c+8}. Shipping each core its inputs ROLLED left by
    512*c columns makes the tile pattern identical on every core (SPMD):
    rows A = local cols [0,512) x local cts 0..8 (ct0 = diagonal block),
    rows B = local cols [4096,4608) x local cts 8..15 (ct8 = diagonal).
    68 [128,512] half-tiles/core vs 128 for the full matrix.
  * bf16 matmuls (fp32 runs at half rate on PE); A = 2*G - sq_i - sq_j with
    -sq_j folded in as matmul row 64 and -sq_i as the exp bias.
  * Reductions: S2 = sum E^2 via DVE scalar_tensor_tensor+accum per tile.
    S1 = sum t_i t_j E_ij: most tiles via PE "matvec" rows (lhsT = t one-hot
    window -> accumulating [17,512] PSUM w rows, one per local column tile;
    final small stt dots w rows with t), a few tiles via DVE stt to balance
    engine load.
"""

import numpy as np
import ml_dtypes

import concourse.bass as bass
import concourse.bacc as bacc
import concourse.tile as tile
import concourse.mybir as mybir
from concourse.bass_utils import run_bass_kernel_spmd

N = 8192
D = 64
NCORES = 8
NB = 16          # 512-row/col blocks
BW = 512         # block width
BIG = 100.0
MULT = mybir.AluOpType.mult

F32 = mybir.dt.float32
BF16 = mybir.dt.bfloat16
BF16NP = ml_dtypes.bfloat16

# unit lists (per rb): (col_start, width, kind); kind: d=diag, w=wide, n=narrow
UNITS_A = [(0, 512, "d"), (512, 1024, "w"), (1536, 1024, "w"),
           (2560, 1024, "w"), (3584, 1024, "w")]
UNITS_B = [(4096, 512, "d"), (4608, 1024, "w"), (5632, 1024, "w"),
           (6656, 1024, "w"), (7680, 512, "n")]


def _ap(tensor, ap, offset=0):
    return bass.AP(tensor=tensor, offset=offset, ap=ap)


def _unit_table():
    """Static flattened unit table: one entry per (rb, unit).
    Returns list of dicts with all per-unit constants."""
    units = []
    uidx = 0          # accumulator slot index (s1acc/s2acc column)
    widx = 0          # wide-unit counter (for the S1 DVE/PE split rule)
    for rb in range(8):
        is_a = rb < 4
        k = rb % 4
        lcol = 128 * k if is_a else 512 + 128 * k
        for (a, w, kind) in (UNITS_A if is_a else UNITS_B):
            s1_dve = False
            if kind == "w":
                s1_dve = (widx % 7 == 3)
                widx += 1
            else:
                s1_dve = True  # narrow + diag units do S1 on DVE
            rows = []
            if not s1_dve:
                for h in range(w // 512):
                    ct = (a + h * 512) // 512
                    if kind == "d" and not is_a:
                        ct = 16
                    rows.append(ct)
            units.append(dict(
                rb=rb, k=k, lcol=lcol, a=a, w=w, kind=kind,
                uidx=uidx, s1_dve=s1_dve, wrows=rows,
                wt=1.0 if kind == "d" else 2.0,
            ))
            uidx += 1
    return units


UNITS = _unit_table()
NUNIT = len(UNITS)           # 40
# host-side weight for each w17 row
WROW_WT = [1.0] + [2.0] * 15 + [1.0]


def build_kernel():
    nc = bacc.Bacc("TRN2", target_bir_lowering=False)

    xb_d = nc.dram_tensor("xb", [D, N], BF16, kind="ExternalInput")
    tb_d = nc.dram_tensor("tb", [N], BF16, kind="ExternalInput")
    tf_d = nc.dram_tensor("tf", [N], F32, kind="ExternalInput")
    params_d = nc.dram_tensor("params", [D], F32, kind="ExternalInput")
    zi_d = nc.dram_tensor("zi", [128, 896], BF16, kind="ExternalInput")
    i128_d = nc.dram_tensor("i128", [128, 128], BF16, kind="ExternalInput")
    rsqb_d = nc.dram_tensor("rsqb_scratch", [N], BF16)
    rsqf_d = nc.dram_tensor("rsqf_scratch", [1024], F32)
    s1o_d = nc.dram_tensor("s1o", [17], F32, kind="ExternalOutput")
    s1ao_d = nc.dram_tensor("s1ao", [128, NUNIT], F32, kind="ExternalOutput")
    s2o_d = nc.dram_tensor("s2o", [128, NUNIT], F32, kind="ExternalOutput")

    with tile.TileContext(nc) as tc:
        with (
            tc.tile_pool(name="const", bufs=1) as cpool,
            tc.tile_pool(name="emm", bufs=2, space="PSUM") as mpool,
            tc.tile_pool(name="wps", bufs=1, space="PSUM") as wpool,
            tc.tile_pool(name="etile", bufs=4) as epool,
            tc.tile_pool(name="scr", bufs=2) as spool,
        ):
            # ---- persistent SBUF ----------------------------------------
            R = cpool.tile([D + 1, N], BF16, tag="R")        # [xb ; -sq]
            tcol = cpool.tile([128, N], BF16, tag="tcol")    # t bcast to 128p
            L = cpool.tile([D + 1, 1024], BF16, tag="L")     # [2p*xb ; ones]
            xb2 = cpool.tile([D, N], BF16, tag="xb2")        # xb*xb
            zi = cpool.tile([128, 896], BF16, tag="zi")
            i128 = cpool.tile([128, 128], BF16, tag="i128")
            WT = cpool.tile([128, 8 * 33], BF16, tag="WT")   # t one-hot wins
            NP = cpool.tile([D, 31], BF16, tag="NP")         # -p one-hot win
            tw = cpool.tile([17, 512], F32, tag="tw")
            trb = cpool.tile([128, 8], BF16, tag="trb")
            trowf = cpool.tile([128, 8], F32, tag="trowf")
            bias = cpool.tile([128, 8], F32, tag="bias")
            psb = cpool.tile([D, 1], F32, tag="psb")
            p2sb = cpool.tile([D, 1], F32, tag="p2sb")
            npf = cpool.tile([D, 1], F32, tag="npf")
            qb = cpool.tile([16, 512], BF16, tag="qb")
            qf = cpool.tile([16, 512], F32, tag="qf")
            s1acc = cpool.tile([128, NUNIT], F32, tag="s1acc")
            s2acc = cpool.tile([128, NUNIT], F32, tag="s2acc")
            s1f = cpool.tile([17, 1], F32, tag="s1f")
            wscr = cpool.tile([17, 512], F32, tag="wscr")
            qsq = wpool.tile([16, 512], F32, tag="qsq")      # -sq chunks
            w17 = wpool.tile([17, 512], F32, tag="w17")      # S1 matvec rows

            # ---- input DMAs (spread across queues) ----------------------
            qs = [nc.sync, nc.gpsimd, nc.scalar, nc.sync]
            for s in range(4):
                sl = slice(s * (N // 4), (s + 1) * (N // 4))
                qs[s].dma_start(out=R[0:D, sl], in_=xb_d[:, sl])
            for s in range(4):
                sl = slice(s * (N // 4), (s + 1) * (N // 4))
                qs[s].dma_start(
                    out=tcol[:, sl],
                    in_=_ap(tb_d, [[0, 128], [1, N // 4]], offset=s * (N // 4)),
                )
            nc.gpsimd.dma_start(out=zi[:, :], in_=zi_d[:, :])
            nc.gpsimd.dma_start(out=i128[:, :], in_=i128_d[:, :])
            nc.gpsimd.dma_start(out=psb[:, :], in_=_ap(params_d, [[1, D], [0, 1]]))
            # t in [128, 4] layouts: rows A (cols 0..512), rows B (4096..4608)
            nc.gpsimd.dma_start(out=trb[:, 0:4], in_=_ap(tb_d, [[1, 128], [128, 4]]))
            nc.gpsimd.dma_start(out=trb[:, 4:8],
                                in_=_ap(tb_d, [[1, 128], [128, 4]], offset=4096))
            nc.gpsimd.dma_start(out=trowf[:, 0:4], in_=_ap(tf_d, [[1, 128], [128, 4]]))
            nc.gpsimd.dma_start(out=trowf[:, 4:8],
                                in_=_ap(tf_d, [[1, 128], [128, 4]], offset=4096))
            nc.gpsimd.dma_start(out=tw[0:16, :], in_=_ap(tf_d, [[512, 16], [1, 512]]))
            nc.gpsimd.dma_start(out=tw[16:17, :],
                                in_=_ap(tf_d, [[0, 1], [1, 512]], offset=4096))

            # ---- small setup compute ------------------------------------
            nc.vector.tensor_scalar_mul(p2sb[:, :], psb[:, :], 2.0)
            nc.vector.tensor_scalar_mul(npf[:, :], psb[:, :], -1.0)
            nc.vector.memset(NP[:, :], 0.0)
            nc.vector.tensor_copy(out=NP[:, 15:16], in_=npf[:, :])
            nc.vector.memset(WT[:, :], 0.0)
            for rb in range(8):
                nc.vector.tensor_copy(out=WT[:, rb * 33 + 16:rb * 33 + 17],
                                      in_=trb[:, rb:rb + 1])
            nc.vector.memset(s1acc[:, :], 0.0)
            # L: [2p*xb ; ones] for the 8 local row blocks (cols 0..512 = A,
            # 512..1024 = B)
            nc.gpsimd.memset(L[D:D + 1, :], 1.0)
            nc.vector.tensor_scalar_mul(L[0:D, 0:512], R[0:D, 0:512], p2sb[:, :])
            nc.vector.tensor_scalar_mul(L[0:D, 512:1024], R[0:D, 4096:4608],
                                        p2sb[:, :])

            # ---- -sq via xb^2 + PE window-matvec reduction --------------
            # xb2 = xb*xb (bf16, 2x DVE) in 4 chunks; 4 matvecs per chunk
            for s in range(4):
                sl = slice(s * (N // 4), (s + 1) * (N // 4))
                nc.vector.tensor_tensor(out=xb2[:, sl], in0=R[0:D, sl],
                                        in1=R[0:D, sl], op=MULT)
                for j in range(4):
                    kk = s * 4 + j
                    nc.tensor.matmul(
                        qsq[0:16, :],
                        NP[:, 15 - kk:31 - kk],
                        xb2[:, kk * 512:(kk + 1) * 512],
                        start=(kk == 0), stop=(kk == 15),
                    )
            # bounce -sq: bf16 row 64 of R (all 16 chunks), f32 bias (local)
            nc.vector.tensor_copy(out=qb[:, :], in_=qsq[:, :])
            nc.sync.dma_start(out=_ap(rsqb_d, [[512, 16], [1, 512]]), in_=qb[:, :])
            for s in range(4):
                sl = slice(s * (N // 4), (s + 1) * (N // 4))
                qs[s].dma_start(out=R[D:D + 1, sl],
                                in_=_ap(rsqb_d, [[0, 1], [1, N // 4]],
                                        offset=s * (N // 4)))
            nc.vector.tensor_copy(out=qf[:, :], in_=qsq[:, :])
            nc.sync.dma_start(out=_ap(rsqf_d, [[1, 512]]), in_=qf[0:1, :])
            nc.sync.dma_start(out=_ap(rsqf_d, [[1, 512]], offset=512), in_=qf[8:9, :])
            nc.sync.dma_start(out=bias[:, 0:4], in_=_ap(rsqf_d, [[1, 128], [128, 4]]))
            nc.sync.dma_start(out=bias[:, 4:8],
                              in_=_ap(rsqf_d, [[1, 128], [128, 4]], offset=512))

            # ---- main loop ----------------------------------------------
            EXP = mybir.ActivationFunctionType.Exp
            n_mv = sum(len(u["wrows"]) for u in UNITS)
            mv_done = 0
            pending = []  # deferred (per previous unit) DVE/PE reduction ops

            def flush_pending():
                nonlocal mv_done
                for fn in pending:
                    mv_done = fn(mv_done)
                pending.clear()

            for u in UNITS:
                w = u["w"]
                mm = mpool.tile([128, w], F32, tag=f"mm{w}")
                for h in range(w // 512):
                    st = True
                    sp = not (u["kind"] == "d" and h == 0)
                    nc.tensor.matmul(
                        mm[:, h * 512:(h + 1) * 512],
                        L[:, u["lcol"]:u["lcol"] + 128],
                        R[:, u["a"] + h * 512:u["a"] + (h + 1) * 512],
                        start=st, stop=sp,
                    )
                if u["kind"] == "d":
                    zoff = 384 - 128 * u["k"]
                    nc.tensor.matmul(
                        mm[:, 0:512], i128[:, :], zi[:, zoff:zoff + 512],
                        start=False, stop=True,
                    )
                E = epool.tile([128, w], BF16, tag=f"E{w}")
                nc.scalar.activation(out=E[:, :], in_=mm[:, :], func=EXP,
                                     bias=bias[:, u["rb"]:u["rb"] + 1], scale=1.0)

                flush_pending()

                def make_ops(u=u, E=E, w=w):
                    def run(mv_done):
                        sc2 = spool.tile([128, w], BF16, tag=f"sc2{w}")
                        nc.vector.scalar_tensor_tensor(
                            out=sc2[:, :], in0=E[:, :], scalar=1.0, in1=E[:, :],
                            op0=MULT, op1=MULT,
                            accum_out=s2acc[:, u["uidx"]:u["uidx"] + 1],
                        )
                        if u["s1_dve"]:
                            sc1 = spool.tile([128, w], BF16, tag=f"sc1{w}")
                            nc.vector.scalar_tensor_tensor(
                                out=sc1[:, :], in0=E[:, :],
                                scalar=trowf[:, u["rb"]:u["rb"] + 1],
                                in1=tcol[:, u["a"]:u["a"] + w],
                                op0=MULT, op1=MULT,
                                accum_out=s1acc[:, u["uidx"]:u["uidx"] + 1],
                            )
                        else:
                            for h, r in enumerate(u["wrows"]):
                                woff = u["rb"] * 33 + 16 - r
                                nc.tensor.matmul(
                                    w17[0:17, :],
                                    WT[:, woff:woff + 17],
                                    E[:, h * 512:(h + 1) * 512],
                                    start=(mv_done == 0),
                                    stop=(mv_done == n_mv - 1),
                                    skip_group_check=True,
                                )
                                mv_done += 1
                        return mv_done
                    return run

                pending.append(make_ops())
            flush_pending()
            assert mv_done == n_mv

            # ---- final reductions + output ------------------------------
            nc.vector.scalar_tensor_tensor(
                out=wscr[:, :], in0=w17[0:17, :], scalar=1.0, in1=tw[:, :],
                op0=MULT, op1=MULT, accum_out=s1f[:, :],
            )
            nc.sync.dma_start(out=_ap(s1o_d, [[1, 17]]), in_=s1f[:, :])
            nc.sync.dma_start(out=s1ao_d[:, :], in_=s1acc[:, :])
            nc.sync.dma_start(out=s2o_d[:, :], in_=s2acc[:, :])

    nc.compile()
    return nc


_NC_CACHE = None


def make_in_maps(X, target, params):
    X = np.ascontiguousarray(X, dtype=np.float32)
    target = np.ascontiguousarray(target, dtype=np.float32)
    params = np.ascontiguousarray(params, dtype=np.float32)
    XT = np.ascontiguousarray(X.T)                       # [64, 8192]

    zi = np.zeros((128, 896), dtype=BF16NP)
    for p in range(128):
        zi[p, 384 + p] = -BIG
    i128 = np.eye(128, dtype=BF16NP)

    in_maps = []
    for c in range(NCORES):
        XTr = np.roll(XT, -BW * c, axis=1)
        tr = np.roll(target, -BW * c)
        in_maps.append({
            "xb": XTr.astype(BF16NP),
            "tb": tr.astype(BF16NP),
            "tf": np.ascontiguousarray(tr),
            "params": params,
            "zi": zi,
            "i128": i128,
        })
    return in_maps


def kernel(X, target, params):
    global _NC_CACHE
    X = np.ascontiguousarray(X, dtype=np.float32)
    target = np.ascontiguousarray(target, dtype=np.float32)
    params = np.ascontiguousarray(params, dtype=np.float32)

    in_maps = make_in_maps(X, target, params)
    if _NC_CACHE is None:
        _NC_CACHE = build_kernel()
    res = run_bass_kernel_spmd(_NC_CACHE, in_maps, core_ids=list(range(NCORES)))

    s1 = float(np.sum(target.astype(np.float64) ** 2))
    s2 = float(N)
    for c in range(NCORES):
        s1o = res.results[c]["s1o"]      # [17]
        s1ao = res.results[c]["s1ao"]    # [128, NUNIT]
        s2o = res.results[c]["s2o"]      # [128, NUNIT]
        for r in range(17):
            s1 += WROW_WT[r] * float(s1o[r])
        for u in UNITS:
            s2 += u["wt"] * float(s2o[:, u["uidx"]].sum())
            if u["s1_dve"]:
                s1 += u["wt"] * float(s1ao[:, u["uidx"]].sum())

    val = -s1 / (N * np.sqrt(s2))
    return np.array(val, dtype=np.float32)


# revision 8
# speedup vs baseline: 2.8258x; 1.0638x over previous
"""Kernel-target-alignment loss on 8 TRN2 NeuronCores (v2).

Math: Xs = X*sqrt(p); d2_ij = ||Xs_i - Xs_j||^2; K = exp(-d2) (diag := 1);
kta = sum(K*tt^T) / (N*sqrt(sum(K*K)));  return -kta.

v2 strategy:
  * Exact diagonal on host: S2 = N + offdiag, S1 = sum(t^2) + offdiag.
    Device computes only off-diagonal sums; the K diagonal is suppressed by
    adding -BIG to A_ii via a second (identity-weights) matmul on diagonal
    tiles, so no bit-exact sq path is needed and everything runs in bf16.
  * Triangle-of-work: by symmetry only ~half the N^2 pairs are computed.
    Row-block r (512 rows) pairs with column blocks r..r+8 (mod 16); core c
    owns row blocks {c, c+8}. Shipping each core its inputs ROLLED left by
    512*c columns makes the tile pattern identical on every core (SPMD):
    rows A = local cols [0,512) x local cts 0..8 (ct0 = diagonal block),
    rows B = local cols [4096,4608) x local cts 8..15 (ct8 = diagonal).
    68 [128,512] half-tiles/core vs 128 for the full matrix.
  * bf16 matmuls (fp32 runs at half rate on PE); A = 2*G - sq_i - sq_j with
    -sq_j folded in as matmul row 64 and -sq_i as the exp bias.
  * Reductions: S2 = sum E^2 via DVE scalar_tensor_tensor+accum per tile.
    S1 = sum t_i t_j E_ij: most tiles via PE "matvec" rows (lhsT = t one-hot
    window -> accumulating [17,512] PSUM w rows, one per local column tile;
    final small stt dots w rows with t), a few tiles via DVE stt to balance
    engine load.
"""

import numpy as np
import ml_dtypes

import concourse.bass as bass
import concourse.bacc as bacc
import concourse.tile as tile
import concourse.mybir as mybir
from concourse.bass_utils import run_bass_kernel_spmd

N = 8192
D = 64
NCORES = 8
NB = 16          # 512-row/col blocks
BW = 512         # block width
BIG = 100.0
MULT = mybir.AluOpType.mult

F32 = mybir.dt.float32
BF16 = mybir.dt.bfloat16
BF16NP = ml_dtypes.bfloat16

# unit lists (per rb): (col_start, width, kind); kind: d=diag, w=wide, n=narrow
UNITS_A = [(0, 512, "d"), (512, 1024, "w"), (1536, 1024, "w"),
           (2560, 1024, "w"), (3584, 1024, "w")]
UNITS_B = [(4096, 512, "d"), (4608, 1024, "w"), (5632, 1024, "w"),
           (6656, 1024, "w"), (7680, 512, "n")]


def _ap(tensor, ap, offset=0):
    return bass.AP(tensor=tensor, offset=offset, ap=ap)


def _unit_table():
    """Static flattened unit table: one entry per (rb, unit).
    Returns list of dicts with all per-unit constants."""
    units = []
    uidx = 0          # accumulator slot index (s1acc/s2acc column)
    widx = 0          # wide-unit counter (for the S1 DVE/PE split rule)
    for rb in range(8):
        is_a = rb < 4
        k = rb % 4
        lcol = 128 * k if is_a else 512 + 128 * k
        for (a, w, kind) in (UNITS_A if is_a else UNITS_B):
            s1_dve = False
            if kind == "w":
                s1_dve = (widx % 7 == 3)
                widx += 1
            else:
                s1_dve = True  # narrow + diag units do S1 on DVE
            rows = []
            if not s1_dve:
                for h in range(w // 512):
                    ct = (a + h * 512) // 512
                    if kind == "d" and not is_a:
                        ct = 16
                    rows.append(ct)
            units.append(dict(
                rb=rb, k=k, lcol=lcol, a=a, w=w, kind=kind,
                uidx=uidx, s1_dve=s1_dve, wrows=rows,
                wt=1.0 if kind == "d" else 2.0,
            ))
            uidx += 1
    return units


UNITS = _unit_table()
NUNIT = len(UNITS)           # 40
# host-side weight for each w17 row
WROW_WT = [1.0] + [2.0] * 15 + [1.0]


def build_kernel():
    nc = bacc.Bacc("TRN2", target_bir_lowering=False)

    xb_d = nc.dram_tensor("xb", [D, N], BF16, kind="ExternalInput")
    tb_d = nc.dram_tensor("tb", [N], BF16, kind="ExternalInput")
    params_d = nc.dram_tensor("params", [D], F32, kind="ExternalInput")
    zp_d = nc.dram_tensor("zp", [128, 1024], BF16, kind="ExternalInput")
    misc_d = nc.dram_tensor("misc", [128, 528], F32, kind="ExternalInput")
    rsqb_d = nc.dram_tensor("rsqb_scratch", [N], BF16)
    rsqf_d = nc.dram_tensor("rsqf_scratch", [1024], F32)
    s1o_d = nc.dram_tensor("s1o", [17], F32, kind="ExternalOutput")
    s1ao_d = nc.dram_tensor("s1ao", [128, NUNIT], F32, kind="ExternalOutput")
    s2o_d = nc.dram_tensor("s2o", [128, NUNIT], F32, kind="ExternalOutput")

    with tile.TileContext(nc) as tc:
        with (
            tc.tile_pool(name="const", bufs=1) as cpool,
            tc.tile_pool(name="emm", bufs=2, space="PSUM") as mpool,
            tc.tile_pool(name="wps", bufs=1, space="PSUM") as wpool,
            tc.tile_pool(name="etile", bufs=4) as epool,
            tc.tile_pool(name="scr", bufs=2) as spool,
        ):
            # ---- persistent SBUF ----------------------------------------
            R = cpool.tile([D + 1, N], BF16, tag="R")        # [xb ; -sq]
            tcol = cpool.tile([128, N], BF16, tag="tcol")    # t bcast to 128p
            L = cpool.tile([D + 1, 1024], BF16, tag="L")     # [2p*xb ; ones]
            xb2 = cpool.tile([D, N], BF16, tag="xb2")        # xb*xb
            zp = cpool.tile([128, 1024], BF16, tag="zp")
            misc = cpool.tile([128, 528], F32, tag="misc")
            WT = cpool.tile([128, 8 * 33], BF16, tag="WT")   # t one-hot wins
            NP = cpool.tile([D, 31], BF16, tag="NP")         # -p one-hot win
            trb = cpool.tile([128, 8], BF16, tag="trb")
            bias = cpool.tile([128, 8], F32, tag="bias")
            psb = cpool.tile([D, 1], F32, tag="psb")
            p2sb = cpool.tile([D, 1], F32, tag="p2sb")
            npf = cpool.tile([D, 1], F32, tag="npf")
            qb = cpool.tile([4, 512], BF16, tag="qb")
            qf = cpool.tile([4, 512], F32, tag="qf")
            s1acc = cpool.tile([128, NUNIT], F32, tag="s1acc")
            s2acc = cpool.tile([128, NUNIT], F32, tag="s2acc")
            s1f = cpool.tile([17, 1], F32, tag="s1f")
            wscr = cpool.tile([17, 512], F32, tag="wscr")
            w17 = wpool.tile([17, 512], F32, tag="w17")      # S1 matvec rows

            # ---- input DMAs + pipelined -sq setup -----------------------
            # R chunk order: cols [0:2048) and [4096:6144) first (they hold
            # the local row blocks that L needs).
            CH = [0, 4096, 2048, 6144]
            qs = [nc.sync, nc.gpsimd, nc.scalar]
            for s, off in enumerate(CH):
                qs[s % 3].dma_start(out=R[0:D, off:off + 2048],
                                    in_=xb_d[:, off:off + 2048])
            nc.scalar.dma_start(out=zp[:, :], in_=zp_d[:, :])
            nc.gpsimd.dma_start(out=psb[:, :], in_=_ap(params_d, [[1, D], [0, 1]]))
            nc.sync.dma_start(out=misc[:, :], in_=misc_d[:, :])
            for s in range(4):
                sl = slice(s * (N // 4), (s + 1) * (N // 4))
                qs[s % 3].dma_start(
                    out=tcol[:, sl],
                    in_=_ap(tb_d, [[0, 128], [1, N // 4]], offset=s * (N // 4)),
                )

            # ---- small setup compute ------------------------------------
            nc.vector.tensor_scalar_mul(p2sb[:, :], psb[:, :], 2.0)
            nc.vector.tensor_scalar_mul(npf[:, :], psb[:, :], -1.0)
            nc.vector.memset(NP[:, :], 0.0)
            nc.vector.tensor_copy(out=NP[:, 15:16], in_=npf[:, :])
            nc.vector.tensor_copy(out=trb[:, :], in_=misc[:, 8:16])
            nc.vector.memset(WT[:, :], 0.0)
            for rb in range(8):
                nc.vector.tensor_copy(out=WT[:, rb * 33 + 16:rb * 33 + 17],
                                      in_=trb[:, rb:rb + 1])
            nc.vector.memset(s1acc[:, :], 0.0)
            # L: [2p*xb ; ones] (cols 0..512 = rows A, 512..1024 = rows B)
            nc.gpsimd.memset(L[D:D + 1, :], 1.0)
            nc.vector.tensor_scalar_mul(L[0:D, 0:512], R[0:D, 0:512], p2sb[:, :])
            nc.vector.tensor_scalar_mul(L[0:D, 512:1024], R[0:D, 4096:4608],
                                        p2sb[:, :])

            # ---- -sq, pipelined in 4 groups of 2048 cols ----------------
            # group g covers cols [g*2048, (g+1)*2048); PSUM [4, 512] rows.
            for g in range(4):
                off = g * 2048
                nc.vector.tensor_tensor(out=xb2[:, off:off + 2048],
                                        in0=R[0:D, off:off + 2048],
                                        in1=R[0:D, off:off + 2048], op=MULT)
                qg = wpool.tile([4, 512], F32, tag="qsqg")
                for j in range(4):
                    nc.tensor.matmul(
                        qg[0:4, :],
                        NP[:, 15 - j:19 - j],
                        xb2[:, off + j * 512:off + (j + 1) * 512],
                        start=(j == 0), stop=(j == 3),
                    )
                nc.vector.tensor_copy(out=qb[:, :], in_=qg[:, :])
                qs[g % 3].dma_start(
                    out=_ap(rsqb_d, [[512, 4], [1, 512]], offset=off),
                    in_=qb[:, :])
                qs[(g + 1) % 3].dma_start(
                    out=R[D:D + 1, off:off + 2048],
                    in_=_ap(rsqb_d, [[0, 1], [1, 2048]], offset=off))
                if g == 0 or g == 2:
                    nc.vector.tensor_copy(out=qf[:, :], in_=qg[:, :])
                    nc.sync.dma_start(
                        out=_ap(rsqf_d, [[1, 512]], offset=(0 if g == 0 else 512)),
                        in_=qf[0:1, :])
            nc.sync.dma_start(out=bias[:, 0:4], in_=_ap(rsqf_d, [[1, 128], [128, 4]]))
            nc.sync.dma_start(out=bias[:, 4:8],
                              in_=_ap(rsqf_d, [[1, 128], [128, 4]], offset=512))

            # ---- main loop ----------------------------------------------
            EXP = mybir.ActivationFunctionType.Exp
            n_mv = sum(len(u["wrows"]) for u in UNITS)
            mv_done = 0
            pending = []  # deferred (per previous unit) DVE/PE reduction ops

            def flush_pending():
                nonlocal mv_done
                for fn in pending:
                    mv_done = fn(mv_done)
                pending.clear()

            for u in UNITS:
                w = u["w"]
                mm = mpool.tile([128, w], F32, tag=f"mm{w}")
                for h in range(w // 512):
                    st = True
                    sp = not (u["kind"] == "d" and h == 0)
                    nc.tensor.matmul(
                        mm[:, h * 512:(h + 1) * 512],
                        L[:, u["lcol"]:u["lcol"] + 128],
                        R[:, u["a"] + h * 512:u["a"] + (h + 1) * 512],
                        start=st, stop=sp,
                    )
                if u["kind"] == "d":
                    zoff = 384 - 128 * u["k"]
                    nc.tensor.matmul(
                        mm[:, 0:512], zp[:, 896:1024], zp[:, zoff:zoff + 512],
                        start=False, stop=True,
                    )
                E = epool.tile([128, w], BF16, tag=f"E{w}")
                nc.scalar.activation(out=E[:, :], in_=mm[:, :], func=EXP,
                                     bias=bias[:, u["rb"]:u["rb"] + 1], scale=1.0)

                flush_pending()

                def make_ops(u=u, E=E, w=w):
                    def run(mv_done):
                        sc2 = spool.tile([128, w], BF16, tag=f"sc2{w}")
                        nc.vector.scalar_tensor_tensor(
                            out=sc2[:, :], in0=E[:, :], scalar=1.0, in1=E[:, :],
                            op0=MULT, op1=MULT,
                            accum_out=s2acc[:, u["uidx"]:u["uidx"] + 1],
                        )
                        if u["s1_dve"]:
                            sc1 = spool.tile([128, w], BF16, tag=f"sc1{w}")
                            nc.vector.scalar_tensor_tensor(
                                out=sc1[:, :], in0=E[:, :],
                                scalar=misc[:, u["rb"]:u["rb"] + 1],
                                in1=tcol[:, u["a"]:u["a"] + w],
                                op0=MULT, op1=MULT,
                                accum_out=s1acc[:, u["uidx"]:u["uidx"] + 1],
                            )
                        else:
                            for h, r in enumerate(u["wrows"]):
                                woff = u["rb"] * 33 + 16 - r
                                nc.tensor.matmul(
                                    w17[0:17, :],
                                    WT[:, woff:woff + 17],
                                    E[:, h * 512:(h + 1) * 512],
                                    start=(mv_done == 0),
                                    stop=(mv_done == n_mv - 1),
                                    skip_group_check=True,
                                )
                                mv_done += 1
                        return mv_done
                    return run

                pending.append(make_ops())
            flush_pending()
            assert mv_done == n_mv

            # ---- final reductions + output ------------------------------
            nc.vector.scalar_tensor_tensor(
                out=wscr[:, :], in0=w17[0:17, :], scalar=1.0,
                in1=misc[0:17, 16:528],
                op0=MULT, op1=MULT, accum_out=s1f[:, :],
            )
            nc.sync.dma_start(out=_ap(s1o_d, [[1, 17]]), in_=s1f[:, :])
            nc.sync.dma_start(out=s1ao_d[:, :], in_=s1acc[:, :])
            nc.sync.dma_start(out=s2o_d[:, :], in_=s2acc[:, :])

    nc.compile()
    return nc


_NC_CACHE = None


def make_in_maps(X, target, params):
    X = np.ascontiguousarray(X, dtype=np.float32)
    target = np.ascontiguousarray(target, dtype=np.float32)
    params = np.ascontiguousarray(params, dtype=np.float32)
    XT = np.ascontiguousarray(X.T)                       # [64, 8192]

    zp = np.zeros((128, 1024), dtype=BF16NP)
    for p in range(128):
        zp[p, 384 + p] = -BIG
        zp[p, 896 + p] = 1.0

    in_maps = []
    for c in range(NCORES):
        XTr = np.roll(XT, -BW * c, axis=1)
        tr = np.roll(target, -BW * c)
        trbv = np.zeros((128, 8), dtype=np.float32)
        trbv[:, 0:4] = tr[0:512].reshape(4, 128).T
        trbv[:, 4:8] = tr[4096:4608].reshape(4, 128).T
        misc = np.zeros((128, 528), dtype=np.float32)
        misc[:, 0:8] = trbv
        misc[:, 8:16] = trbv
        misc[0:16, 16:528] = tr.reshape(16, 512)
        misc[16, 16:528] = tr[4096:4608]
        in_maps.append({
            "xb": XTr.astype(BF16NP),
            "tb": tr.astype(BF16NP),
            "params": params,
            "zp": zp,
            "misc": misc,
        })
    return in_maps


def kernel(X, target, params):
    global _NC_CACHE
    X = np.ascontiguousarray(X, dtype=np.float32)
    target = np.ascontiguousarray(target, dtype=np.float32)
    params = np.ascontiguousarray(params, dtype=np.float32)

    in_maps = make_in_maps(X, target, params)
    if _NC_CACHE is None:
        _NC_CACHE = build_kernel()
    res = run_bass_kernel_spmd(_NC_CACHE, in_maps, core_ids=list(range(NCORES)))

    s1 = float(np.sum(target.astype(np.float64) ** 2))
    s2 = float(N)
    for c in range(NCORES):
        s1o = res.results[c]["s1o"]      # [17]
        s1ao = res.results[c]["s1ao"]    # [128, NUNIT]
        s2o = res.results[c]["s2o"]      # [128, NUNIT]
        for r in range(17):
            s1 += WROW_WT[r] * float(s1o[r])
        for u in UNITS:
            s2 += u["wt"] * float(s2o[:, u["uidx"]].sum())
            if u["s1_dve"]:
                s1 += u["wt"] * float(s1ao[:, u["uidx"]].sum())

    val = -s1 / (N * np.sqrt(s2))
    return np.array(val, dtype=np.float32)
